# revision 19
# baseline (speedup 1.0000x reference)
"""MultiHeadLatentAttention Trainium2 Bass kernel (optimized).

Sharding (8 cores): core c = (b, hg) with b = c // 2, hg = c % 2.
Each core handles batch b and head-group hg (8 of 16 heads):
  - QKV projection for its heads (weights pre-sliced+transposed+bf16 on host)
  - qk rmsnorm + RoPE + causal attention for its 8 heads
  - pairwise AllGather of y^T (bf16) between (2b, 2b+1)
  - out-projection for c-half hg*1024:(hg+1)*1024 with the full 16 heads
Key optimizations vs v1:
  - bf16 for all matmul operands (x^T, w, q^T, k^T, v, probs, y, w_out);
    PSUM accumulation stays fp32.
  - attention: one [128,1024] PSUM score tile per s-chunk covering both
    heads of the pair; ONE exp activation + ONE mask multiply per chunk.
  - software-pipelined attention inner loop (QK of chunk i+1 issued before
    PV of chunk i) so the scalar-engine exp hides under PE work.
  - rmsnorm stats on Pool(sq)+DVE(reduce), rope on Pool, scale-apply on DVE
    with bf16 output; fewer/larger DMAs.
"""

import numpy as np

import concourse.bass as bass
import concourse.mybir as mybir
import concourse.tile as tile
from concourse import bacc
from concourse.bass import ts
from concourse.masks import make_identity

F32 = mybir.dt.float32
F32R = mybir.dt.float32r
BF16 = mybir.dt.bfloat16

N_HEAD = 16
N_EMBD = 2048
N_LATENT = 1024
HEAD_DIM = 64
ROPE_BASE = 10000.0
EPS = 1e-6
N_CORES = 8

HPC = N_HEAD // 2        # heads per core = 8
DW = HPC * HEAD_DIM      # local head width = 512
TCH = 512                # t-chunk for attention moving dim


def build_nc(T=2048, C=2048, num_devices=N_CORES):
    """Build the SPMD program (identical on all cores; data differs)."""
    nc = bacc.Bacc("TRN2", target_bir_lowering=False, debug=False,
                   num_devices=num_devices)

    NT = T // 128            # t-tiles
    NCT = C // 128           # c-tiles (contraction tiles for qkv proj)
    NJ = T // TCH            # t-chunks for attention
    CH = C // 2              # out c-half width = 1024
    NL = N_LATENT // 128     # l-tiles for out proj = 8
    CCW = 512                # out column chunk
    NCC = CH // CCW

    xT_d = nc.dram_tensor("xT", [C, T], BF16, kind="ExternalInput").ap()
    wqT_d = nc.dram_tensor("wqT", [C, DW], BF16, kind="ExternalInput").ap()
    wkT_d = nc.dram_tensor("wkT", [C, DW], BF16, kind="ExternalInput").ap()
    wvT_d = nc.dram_tensor("wvT", [C, DW], BF16, kind="ExternalInput").ap()
    woT_d = nc.dram_tensor("woutT", [N_LATENT, CH], BF16, kind="ExternalInput").ap()
    cos_d = nc.dram_tensor("cosf", [T, DW], F32, kind="ExternalInput").ap()
    sin_d = nc.dram_tensor("sinf", [T, DW], F32, kind="ExternalInput").ap()
    mask_d = nc.dram_tensor("masks", [4, 128, 2 * TCH], BF16,
                            kind="ExternalInput").ap()
    out_d = nc.dram_tensor("out_half", [T, CH], F32, kind="ExternalOutput").ap()

    groups = [[i, i + 1] for i in range(0, num_devices, 2)]

    with tile.TileContext(nc) as tc:
        with (
            tc.tile_pool(name="const", bufs=1) as constp,
            tc.tile_pool(name="dram", bufs=1, space=bass.MemorySpace.DRAM) as dramp,
        ):
            ident = constp.tile([128, 128], F32, tag="ident")
            make_identity(nc, ident[:])
            identr = constp.tile([128, 128], F32R, tag="identr")
            nc.vector.tensor_copy(identr[:], ident[:])
            identb = constp.tile([128, 128], BF16, tag="identb")
            nc.vector.tensor_copy(identb[:], ident[:])
            eps_sb = constp.tile([128, 1], F32, tag="eps")
            nc.vector.memset(eps_sb[:], EPS)
            ones8 = constp.tile([128, HPC], BF16, tag="ones8")
            nc.vector.memset(ones8[:], 1.0)
            ones_f = constp.tile([128, 64], F32, tag="ones_f")
            nc.vector.memset(ones_f[:], 1.0)
            onesr = constp.tile([128, 64], F32R, tag="onesr")
            nc.vector.tensor_copy(onesr[:], ones_f[:])
            mask_sb = []
            for o in range(4):
                m = constp.tile([128, 2 * TCH], BF16, tag=f"mask{o}",
                                name=f"mask{o}")
                nc.sync.dma_start(m[:], mask_d[o])
                mask_sb.append(m)

            qtd = dramp.tile([DW, T], BF16, tag="qtd")
            ktd = dramp.tile([DW, T], BF16, tag="ktd")
            vd = dramp.tile([T, DW], BF16, tag="vd")
            ytl = dramp.tile([DW, T], BF16, tag="ytl")
            ytfs = []
            for hp in range(HPC // 2):
                yf = dramp.tile([256, T], BF16, tag=f"ytf{hp}", name=f"ytf{hp}")
                ytfs.append(yf)

            # ---------------- Phase 1: QKV + rmsnorm + rope + transpose ----
            with (
                tc.tile_pool(name="p1w", bufs=1) as p1w,
                tc.tile_pool(name="p1", bufs=2) as p1,
                tc.tile_pool(name="p1qk", bufs=2, space=bass.MemorySpace.PSUM) as p1qk,
                tc.tile_pool(name="p1v", bufs=2, space=bass.MemorySpace.PSUM) as p1v,
                tc.tile_pool(name="p1tp", bufs=2, space=bass.MemorySpace.PSUM) as p1tp,
            ):
                wsb = {}
                for name, wd in (("q", wqT_d), ("k", wkT_d), ("v", wvT_d)):
                    w = p1w.tile([128, NCT * DW], BF16, tag=f"w{name}",
                                 name=f"w{name}")
                    nc.sync.dma_start(
                        w[:].rearrange("p (ct d) -> p ct d", d=DW),
                        wd.rearrange("(ct p) d -> p ct d", p=128),
                    )
                    wsb[name] = w

                xTv = xT_d.rearrange("(ct p) t -> p ct t", p=128)
                for tt in range(NT):
                    cos_t = p1.tile([128, DW], F32, tag="cos")
                    sin_t = p1.tile([128, DW], F32, tag="sin")
                    nc.sync.dma_start(cos_t[:], cos_d[ts(tt, 128), :])
                    nc.sync.dma_start(sin_t[:], sin_d[ts(tt, 128), :])

                    # x^T tile [c, 128t] as NCT column blocks, straight from
                    # the host-transposed input (no on-chip transposes)
                    xt = p1.tile([128, NCT * 128], BF16, tag="xt")
                    nc.sync.dma_start(
                        xt[:].rearrange("p (ct t) -> p ct t", t=128),
                        xTv[:, :, ts(tt, 128)],
                    )

                    # qk into one [128,1024] psum (q cols 0:512, k 512:1024)
                    pqk = p1qk.tile([128, 1024], F32, tag="pqk")
                    pv = p1v.tile([128, DW], F32, tag="pv")
                    for ct in range(NCT):
                        nc.tensor.matmul(
                            pqk[:, 0:DW], xt[:, ts(ct, 128)],
                            wsb["q"][:, ts(ct, DW)],
                            start=(ct == 0), stop=(ct == NCT - 1),
                        )
                    for ct in range(NCT):
                        nc.tensor.matmul(
                            pqk[:, DW:2 * DW], xt[:, ts(ct, 128)],
                            wsb["k"][:, ts(ct, DW)],
                            start=(ct == 0), stop=(ct == NCT - 1),
                        )
                    for ct in range(NCT):
                        nc.tensor.matmul(
                            pv[:], xt[:, ts(ct, 128)],
                            wsb["v"][:, ts(ct, DW)],
                            start=(ct == 0), stop=(ct == NCT - 1),
                        )

                    # V: evacuate (cast bf16) to DRAM
                    vsb = p1.tile([128, DW], BF16, tag="vsb", bufs=4)
                    nc.scalar.activation(
                        vsb[:], pv[:], mybir.ActivationFunctionType.Copy
                    )
                    nc.sync.dma_start(vd[ts(tt, 128), :], vsb[:])

                    # rmsnorm + rope for q, k
                    for qi, dst in ((0, qtd), (1, ktd)):
                        # evacuate to SBUF fp32
                        qs = p1.tile([128, DW], F32, tag="qs", bufs=3)
                        nc.scalar.activation(
                            qs[:], pqk[:, qi * DW:(qi + 1) * DW],
                            mybir.ActivationFunctionType.Copy,
                        )
                        # sumsq per head: Pool square, DVE segment-reduce
                        sq = p1.tile([128, DW], F32, tag="sq", bufs=3)
                        nc.gpsimd.tensor_mul(sq[:], qs[:], qs[:])
                        ss = p1.tile([128, HPC], F32, tag="ss", bufs=3)
                        nc.vector.tensor_reduce(
                            ss[:],
                            sq[:].rearrange("p (h d) -> p h d", d=HEAD_DIM),
                            axis=mybir.AxisListType.X,
                            op=mybir.AluOpType.add,
                        )
                        sig = p1.tile([128, HPC], F32, tag="sig", bufs=3)
                        nc.scalar.activation(
                            sig[:], ss[:],
                            mybir.ActivationFunctionType.Sqrt,
                            bias=eps_sb[:], scale=1.0 / HEAD_DIM,
                        )
                        rfac = p1.tile([128, HPC], F32, tag="rfac", bufs=3)
                        nc.vector.reciprocal(rfac[:], sig[:])
                        # rope on Pool: m2 = swap(qs)*sinf, m1 = qs*cos, m3 sum
                        m2 = p1.tile([128, DW], F32, tag="m2", bufs=3)
                        qv = qs[:].rearrange("p (h two d) -> p h two d", two=2,
                                             d=HEAD_DIM // 2)
                        m2v = m2[:].rearrange("p (h two d) -> p h two d", two=2,
                                              d=HEAD_DIM // 2)
                        sv = sin_t[:].rearrange("p (h two d) -> p h two d",
                                                two=2, d=HEAD_DIM // 2)
                        nc.gpsimd.tensor_mul(m2v[:, :, 0, :], qv[:, :, 1, :],
                                             sv[:, :, 0, :])
                        nc.gpsimd.tensor_mul(m2v[:, :, 1, :], qv[:, :, 0, :],
                                             sv[:, :, 1, :])
                        m1 = p1.tile([128, DW], F32, tag="m1", bufs=3)
                        nc.gpsimd.tensor_mul(m1[:], qs[:], cos_t[:])
                        m3 = p1.tile([128, DW], F32, tag="m3", bufs=3)
                        nc.gpsimd.tensor_add(m3[:], m1[:], m2[:])
                        # apply rmsnorm scale per head; cast to bf16
                        qrb = p1.tile([128, DW], BF16, tag="qrb", bufs=3)
                        for h in range(HPC):
                            nc.vector.tensor_scalar_mul(
                                qrb[:, ts(h, HEAD_DIM)],
                                m3[:, ts(h, HEAD_DIM)],
                                rfac[:, h: h + 1],
                            )
                        # transpose to [d, t] (bf16) and store with one DMA
                        tps = p1tp.tile([128, 512], BF16, tag="tps")
                        for db in range(DW // 128):
                            nc.tensor.transpose(
                                tps[:, ts(db, 128)], qrb[:, ts(db, 128)],
                                identb[:]
                            )
                        qt = p1.tile([128, DW], BF16, tag="qt", bufs=4)
                        nc.vector.tensor_copy(qt[:], tps[:])
                        nc.sync.dma_start(
                            dst[:, ts(tt, 128)].rearrange(
                                "(db p) t -> p db t", p=128),
                            qt[:].rearrange("p (db t) -> p db t", t=128),
                        )

            # ---------------- Phase 2: attention --------------------------
            with (
                tc.tile_pool(name="p2kv", bufs=1) as p2kv,
                tc.tile_pool(name="p2", bufs=4) as p2,
                tc.tile_pool(name="p2s", bufs=2, space=bass.MemorySpace.PSUM) as p2s,
                tc.tile_pool(name="p2y", bufs=1, space=bass.MemorySpace.PSUM) as p2y,
                tc.tile_pool(name="p2bc", bufs=2, space=bass.MemorySpace.PSUM) as p2bc,
            ):
                kts = []
                for hp in range(HPC // 2):
                    kt = p2kv.tile([128, T], BF16, tag=f"kt{hp}", name=f"kt{hp}")
                    nc.sync.dma_start(kt[:], ktd[ts(hp, 128), :])
                    kts.append(kt)
                v65 = []
                for si in range(NT):
                    v = p2kv.tile([128, HPC * 65], BF16, tag=f"v65_{si}",
                                  name=f"v65_{si}")
                    vv = v[:].rearrange("p (h e) -> p h e", e=65)
                    nc.vector.tensor_copy(
                        vv[:, :, 64:65].rearrange("p h one -> p (h one)"),
                        ones8[:])
                    nc.sync.dma_start(vv[:, :, 0:64], vd[ts(si, 128), :]
                                      .rearrange("p (h d) -> p h d", d=HEAD_DIM))
                    v65.append(v)

                scale = 1.0 / np.sqrt(HEAD_DIM)

                for hp in range(HPC // 2):
                    for j in range(NJ):
                        q2 = p2.tile([128, TCH], BF16, tag="q2", bufs=2)
                        nc.sync.dma_start(q2[:], qtd[ts(hp, 128), ts(j, TCH)])
                        smax = (j + 1) * (TCH // 128)
                        pys = []
                        for e in range(2):
                            pys.append(p2y.tile([65, TCH], F32, tag=f"py{e}",
                                                name=f"py{e}"))

                        def score(si):
                            pss = p2s.tile([128, 2 * TCH], F32, tag="pss")
                            for e in range(2):
                                nc.tensor.matmul(
                                    pss[:, ts(e, TCH)],
                                    kts[hp][64 * e: 64 * e + 64, ts(si, 128)],
                                    q2[64 * e: 64 * e + 64, :],
                                )
                            pt = p2.tile([128, 2 * TCH], BF16, tag="pt", bufs=3)
                            nc.scalar.activation(
                                pt[:], pss[:],
                                mybir.ActivationFunctionType.Exp,
                                scale=scale,
                            )
                            o = si - (smax - TCH // 128)
                            if o >= 0:
                                nc.vector.tensor_mul(pt[:], pt[:], mask_sb[o][:])
                            return pt

                        pts = {0: score(0)}
                        for si in range(smax):
                            if si + 1 < smax:
                                pts[si + 1] = score(si + 1)
                            pt = pts.pop(si)
                            for e in range(2):
                                h = 2 * hp + e
                                nc.tensor.matmul(
                                    pys[e][:],
                                    v65[si][:, 65 * h: 65 * h + 65],
                                    pt[:, ts(e, TCH)],
                                    start=(si == 0),
                                    stop=(si == smax - 1),
                                )
                        ynt = p2.tile([128, TCH], BF16, tag="ynt", bufs=2)
                        for e in range(2):
                            ystage = p2.tile([65, TCH], F32R, tag="ystage",
                                             bufs=2)
                            nc.vector.tensor_copy(ystage[:], pys[e][:])
                            bc = p2bc.tile([64, TCH], F32, tag="bc")
                            nc.tensor.matmul(
                                bc[:], onesr[64:65, :], ystage[64:65, :]
                            )
                            bcr = p2.tile([64, TCH], F32, tag="bcr", bufs=2)
                            nc.vector.reciprocal(bcr[:], bc[:])
                            nc.vector.tensor_mul(
                                ynt[64 * e: 64 * e + 64, :],
                                ystage[0:64, :], bcr[:]
                            )
                        nc.sync.dma_start(
                            ytl[ts(hp, 128), ts(j, TCH)], ynt[:]
                        )
                    nc.gpsimd.collective_compute(
                        "AllGather",
                        mybir.AluOpType.bypass,
                        replica_groups=groups,
                        ins=[ytl[ts(hp, 128), :]],
                        outs=[ytfs[hp][:]],
                    )

            # ---------------- Phase 3: out projection ---------------------
            with (
                tc.tile_pool(name="p3w", bufs=1) as p3w,
                tc.tile_pool(name="p3", bufs=3) as p3,
                tc.tile_pool(name="p3y", bufs=1) as p3y,
                tc.tile_pool(name="p3ps", bufs=3, space=bass.MemorySpace.PSUM) as p3ps,
            ):
                wo = p3w.tile([128, NL * CH], BF16, tag="wo")
                nc.sync.dma_start(
                    wo[:].rearrange("p (lt c) -> p lt c", c=CH),
                    woT_d.rearrange("(lt p) c -> p lt c", p=128),
                )
                lt_order = [0, 4, 1, 5, 2, 6, 3, 7]  # AG-arrival order
                yts = {}
                for lt in lt_order:
                    y = p3y.tile([128, T], BF16, tag=f"yr{lt}", name=f"yr{lt}")
                    nc.sync.dma_start(
                        y[:], ytfs[lt % 4][(lt // 4) * 128:(lt // 4 + 1) * 128, :])
                    yts[lt] = y
                for tt in range(NT):
                    for cc in range(NCC):
                        po = p3ps.tile([128, CCW], F32, tag="po")
                        for i, lt in enumerate(lt_order):
                            nc.tensor.matmul(
                                po[:],
                                yts[lt][:, ts(tt, 128)],
                                wo[:, lt * CH + cc * CCW: lt * CH + (cc + 1) * CCW],
                                start=(i == 0),
                                stop=(i == NL - 1),
                            )
                        osb = p3.tile([128, CCW], F32, tag="osb")
                        nc.scalar.activation(
                            osb[:], po[:], mybir.ActivationFunctionType.Copy
                        )
                        nc.sync.dma_start(
                            out_d[ts(tt, 128), ts(cc, CCW)], osb[:]
                        )

    nc.compile()
    return nc


def host_tables(T=2048):
    inv_freq = 1.0 / (ROPE_BASE ** (np.arange(0, HEAD_DIM, 2, dtype=np.float32)
                                    / HEAD_DIM))
    t = np.arange(T, dtype=np.float32)
    freqs = np.outer(t, inv_freq)
    cos = np.cos(freqs).astype(np.float32)
    sin = np.sin(freqs).astype(np.float32)
    cosf = np.tile(np.concatenate([cos, cos], axis=1), (1, HPC))
    sinf = np.tile(np.concatenate([sin, -sin], axis=1), (1, HPC))
    masks = np.zeros((4, 128, TCH), dtype=np.float32)
    for i, o in enumerate(range(0, TCH, 128)):
        masks[i] = (np.arange(TCH)[None, :] >=
                    (np.arange(128)[:, None] + o)).astype(np.float32)
    masks2 = np.concatenate([masks, masks], axis=2)  # same mask for 2 heads
    return np.ascontiguousarray(cosf), np.ascontiguousarray(sinf), masks2


def make_in_maps(x, w_qkv, w_out, T=2048, num_devices=N_CORES):
    from ml_dtypes import bfloat16

    x = np.asarray(x, dtype=np.float32)
    w_qkv = np.asarray(w_qkv, dtype=np.float32)
    w_out = np.asarray(w_out, dtype=np.float32)
    C = x.shape[-1]
    cosf, sinf, masks = host_tables(T)
    masks_b = masks.astype(bfloat16)
    in_maps = []
    for c in range(num_devices):
        b, hg = c // 2, c % 2
        sl = slice(hg * DW, (hg + 1) * DW)
        in_maps.append({
            "xT": np.ascontiguousarray(x[b].T.astype(bfloat16)),
            "wqT": np.ascontiguousarray(
                w_qkv[0 * N_LATENT:, :][sl].T.astype(bfloat16)),
            "wkT": np.ascontiguousarray(
                w_qkv[1 * N_LATENT:, :][sl].T.astype(bfloat16)),
            "wvT": np.ascontiguousarray(
                w_qkv[2 * N_LATENT:, :][sl].T.astype(bfloat16)),
            "woutT": np.ascontiguousarray(
                w_out[hg * C // 2:(hg + 1) * C // 2, :].T.astype(bfloat16)),
            "cosf": cosf,
            "sinf": sinf,
            "masks": masks_b,
        })
    return in_maps


_NC = None


def kernel(x, w_qkv, w_out):
    global _NC
    if _NC is None:
        _NC = build_nc()
    from concourse.bass_utils import run_bass_kernel_spmd
    in_maps = make_in_maps(x, w_qkv, w_out)
    res = run_bass_kernel_spmd(_NC, in_maps, list(range(N_CORES))).results
    B, T = 4, 2048
    out = np.empty((B, T, N_EMBD), dtype=np.float32)
    for c in range(N_CORES):
        b, hg = c // 2, c % 2
        out[b, :, hg * N_EMBD // 2:(hg + 1) * N_EMBD // 2] = res[c]["out_half"]
    return out
